# revision 35
# baseline (speedup 1.0000x reference)
"""Minibatch discrimination 1d kernel for TRN2, 8 NeuronCores.

Reference computation:
    M = (x @ T.reshape(A, B*C)).reshape(N, B, C)        # N=512, A=1024, B=64, C=16
    l1[i,j,b] = sum_c |M[i,b,c] - M[j,b,c]|
    out[i,b]  = sum_j exp(-l1[i,j,b]) - 1
    return concat([x, out], axis=1)                     # (512, 1088)

Distribution: shard the i (row) axis across 8 cores (64 rows each); every core
computes the full M^T = (B*C, N) on-chip (replicated preamble matmul) and the
pairwise reduction for its own rows against all N columns.

Per-core on-chip dataflow, for each of the 64 rows i:
  - DVE:  8x tensor_scalar(|M^T_blk - m_i|) in bf16 (4x perf mode)
  - PE:   8x block-diagonal ones-matmuls reduce the 16 C-lanes per B into
          PSUM l1[64b, 512j] (fp32 accumulate)
  - ACT:  exp(-l1) with free-axis accum_out -> rowsum[64b, 1]
bf16 is numerically safe here: l1 >= ~135 off-diagonal so exp(-l1) underflows
against the diagonal term 1.0 in fp32 regardless of small l1 perturbations.
"""

import os
import sys

import numpy as np

for _p in ("/opt/trn_rl_repo",):
    if _p not in sys.path:
        sys.path.insert(0, _p)

import ml_dtypes  # noqa: E402
from concourse import bass, tile  # noqa: E402
from concourse import mybir  # noqa: E402
from concourse.bass_utils import run_bass_kernel_spmd  # noqa: E402

N, A, B, C = 512, 1024, 64, 16
NCORES = 8
ROWS = N // NCORES  # 64 rows per core
BC = B * C  # 1024
NBLK = BC // 128  # 8 partition blocks of M^T
ABLK = A // 128  # 8 contraction blocks
OUTW = A + B  # 1088
CHUNK = 3  # rows per stationary-reuse chunk (2*CHUNK PSUM banks in flight)

F32 = mybir.dt.float32
BF16 = mybir.dt.bfloat16


def build_nc():
    nc = bass.Bass()
    xr_d = nc.declare_dram_parameter("xr", [ROWS, A], F32, isOutput=False)
    xrt_d = nc.declare_dram_parameter("xrt", [A, ROWS], F32, isOutput=False)
    xt_d = nc.declare_dram_parameter("xt", [A, N], F32, isOutput=False)
    t2_d = nc.declare_dram_parameter("t2", [A, BC], F32, isOutput=False)
    wg_d = nc.declare_dram_parameter("wg", [128, NBLK * B], BF16, isOutput=False)
    id_d = nc.declare_dram_parameter("id64", [B, B], F32, isOutput=False)
    out_d = nc.declare_dram_parameter("out", [ROWS, OUTW], F32, isOutput=True)

    with tile.TileContext(nc) as tc:
        with (
            tc.tile_pool(name="const", bufs=1) as cpool,
            tc.tile_pool(name="work", bufs=2 * CHUNK) as wpool,
            tc.tile_pool(name="pl1", bufs=2 * CHUNK, space="PSUM") as pl1,
        ):
            # ---- output staging: x rows land in SBUF, cols appended later,
            # one single DMA writes the full [64, 1088] block ----
            out_sb = cpool.tile([ROWS, OUTW], F32)
            nc.sync.dma_start(out=out_sb[:, 0:A], in_=xr_d[:])
            # in-place touches: absorb the xr-load DMA wait into ACT's clock
            # and order the final output DMA after it (single wait slot).
            # The second touch carries no wait of its own -- it is a spare
            # wait slot for _legalize_waits to park the output DMA's
            # queue-ring wait on.
            nc.scalar.copy(out_sb[:, 0:1], out_sb[:, 0:1])
            nc.scalar.copy(out_sb[:, 1:2], out_sb[:, 1:2])

            # ---- load + cast inputs ----
            t2_f = cpool.tile([128, ABLK, BC], F32)
            t2_b = cpool.tile([128, ABLK, BC], BF16)
            xt_f = cpool.tile([128, ABLK, N], F32)
            xt_b = cpool.tile([128, ABLK, N], BF16)
            xrt_f = cpool.tile([128, ABLK, ROWS], F32)
            xrt_b = cpool.tile([128, ABLK, ROWS], BF16)
            wg_s = cpool.tile([128, NBLK * B], BF16)
            id_s = cpool.tile([B, B], F32)

            nc.sync.dma_start(out=wg_s[:], in_=wg_d[:])
            nc.sync.dma_start(out=id_s[:], in_=id_d[:])
            for k in range(ABLK):
                nc.sync.dma_start(out=t2_f[:, k, :], in_=t2_d[128 * k : 128 * (k + 1), :])
                nc.sync.dma_start(out=xt_f[:, k, :], in_=xt_d[128 * k : 128 * (k + 1), :])
                nc.sync.dma_start(out=xrt_f[:, k, :], in_=xrt_d[128 * k : 128 * (k + 1), :])
            for k in range(ABLK):
                nc.vector.tensor_copy(t2_b[:, k, :], t2_f[:, k, :])
                nc.vector.tensor_copy(xt_b[:, k, :], xt_f[:, k, :])
                nc.vector.tensor_copy(xrt_b[:, k, :], xrt_f[:, k, :])

            # ---- M^T = t2^T @ x^T : [BC, N] in 8 partition blocks ----
            mt_b = cpool.tile([128, NBLK, N], BF16)
            mi_h = cpool.tile([128, NBLK, ROWS], BF16)
            mi_b = cpool.tile([128, NBLK, ROWS], F32)
            mi_n = cpool.tile([128, NBLK, ROWS], F32)  # negated, ACT abs bias
            with tc.tile_pool(name="ppre", bufs=1, space="PSUM") as ppre:
              for g in range(NBLK):
                  pm = ppre.tile([128, N], F32, tag="pmt")
                  for k in range(ABLK):
                      nc.tensor.matmul(
                          pm[:],
                          t2_b[:, k, 128 * g : 128 * (g + 1)],
                          xt_b[:, k, :],
                          start=(k == 0),
                          stop=(k == ABLK - 1),
                      )
                  # split psum->sbuf cast copies between ACT and DVE
                  if g % 2 == 0:
                      nc.scalar.copy(mt_b[:, g, :], pm[:])
                  else:
                      nc.vector.tensor_copy(mt_b[:, g, :], pm[:])
                  pmi = ppre.tile([128, ROWS], F32, tag="pmi")
                  for k in range(ABLK):
                      nc.tensor.matmul(
                          pmi[:],
                          t2_b[:, k, 128 * g : 128 * (g + 1)],
                          xrt_b[:, k, :],
                          start=(k == 0),
                          stop=(k == ABLK - 1),
                      )
                  # round mi through bf16 so the diagonal |mt_b[:, i] - mi| is
                  # exactly zero (mt_b is bf16-rounded)
                  nc.vector.tensor_copy(mi_h[:, g, :], pmi[:])
                  nc.vector.tensor_copy(mi_b[:, g, :], mi_h[:, g, :])
                  nc.vector.tensor_scalar(
                      mi_n[:, g, :], mi_b[:, g, :], -1.0, None, op0=mybir.AluOpType.mult
                  )

            # ---- main pairwise loop over this core's rows ----
            # |mt - mi| per block: DVE handles ND blocks as subtract (4x perf
            # mode) + one batched uint16 sign-clear AND (4x); ACT handles the
            # rest as a single Abs(x + (-mi)) activation pass per block.
            # Rows are processed in chunks of CHUNK; within a chunk the
            # reduction matmuls run g-outer so each stationary W_g is loaded
            # once per chunk instead of once per matmul (LDWEIGHTS amortize).
            ND = int(os.environ.get("KERNEL_ND", "6"))
            rowacc = cpool.tile([B, ROWS], F32)
            for i0 in range(0, ROWS, CHUNK):
                csz = min(CHUNK, ROWS - i0)
                dvs, das, l1s = [], [], []
                for i in range(i0, i0 + csz):
                    dv = wpool.tile([128, ND, N], BF16, tag="dv")
                    da = None
                    if ND < NBLK:
                        da = wpool.tile([128, NBLK - ND, N], BF16, tag="da")
                    for g in range(ND):
                        nc.vector.tensor_scalar(
                            dv[:, g, :],
                            mt_b[:, g, :],
                            mi_b[:, g, i : i + 1],
                            None,
                            op0=mybir.AluOpType.subtract,
                        )
                    nc.vector.tensor_scalar(
                        dv[:].bitcast(mybir.dt.uint16),
                        dv[:].bitcast(mybir.dt.uint16),
                        0x7FFF,
                        None,
                        op0=mybir.AluOpType.bitwise_and,
                    )
                    for g in range(ND, NBLK):
                        nc.scalar.activation(
                            da[:, g - ND, :],
                            mt_b[:, g, :],
                            mybir.ActivationFunctionType.Abs,
                            bias=mi_n[:, g, i : i + 1],
                            scale=1.0,
                        )
                    dvs.append(dv)
                    das.append(da)
                    l1 = pl1.tile([B, N], F32, tag="l1")
                    l1s.append(l1)
                for g in range(NBLK):
                    for ic in range(csz):
                        src = dvs[ic][:, g, :] if g < ND else das[ic][:, g - ND, :]
                        nc.tensor.matmul(
                            l1s[ic][:],
                            wg_s[:, B * g : B * (g + 1)],
                            src,
                            start=(g == 0),
                            stop=(g == NBLK - 1),
                        )
                for ic in range(csz):
                    i = i0 + ic
                    et = wpool.tile([B, N], BF16, tag="e")
                    nc.scalar.activation(
                        et[:],
                        l1s[ic][:],
                        mybir.ActivationFunctionType.Exp,
                        scale=-1.0,
                        accum_out=rowacc[:, i : i + 1],
                    )

            # ---- finalize: subtract diagonal term, transpose, store ----
            occ = cpool.tile([B, ROWS], F32)
            nc.vector.tensor_scalar(
                occ[:], rowacc[:], 1.0, None, op0=mybir.AluOpType.subtract
            )
            pt = pl1.tile([ROWS, B], F32, tag="l1")
            nc.tensor.transpose(pt[:], occ[:], id_s[:])
            nc.scalar.copy(out_sb[:, A:OUTW], pt[:])
            nc.scalar.dma_start(out=out_d[:], in_=out_sb[:])

    if not os.environ.get("KERNEL_NO_STRIP"):
        _legalize_waits(nc)
    return nc


def _legalize_waits(nc):
    """walrus rejects compute instructions (TensorScalarPtr / Activation /
    Matmult / ...) with more than one sync-wait: their trn2 ISA structs have
    a single wait slot.  Tile sometimes emits 2.  Two safe rewrites:

    1. Strip self-waits that are provably redundant: the waited semaphore is
       incremented exclusively by instructions of the waiter's own engine
       (verified globally) and the wait value is <= the number of increments
       issued earlier in this engine's strictly in-order stream, so the wait
       is always satisfied by engine ordering alone.

    2. Move an excess wait onto the *immediately preceding* instruction of
       the same engine (typically the Ldweights in front of a Matmult) when
       that instruction has a free wait slot.  Waiting earlier in the same
       in-order stream is strictly more conservative, and since there is no
       same-engine instruction between carrier and target, no foreign
       producer chain can depend on an instruction in that window (deadlock
       impossible).
    """
    from collections import defaultdict

    from concourse import mybir as mb

    # Engine-owned semaphore name prefixes.  DMAHW*/DMASW* sems increment
    # asynchronously at DMA *completion*, not at instruction retirement on
    # the issuing engine, so they are never "self" sems.
    eng_pfx = {
        mb.EngineType.PE: "PE_",
        mb.EngineType.DVE: "DVE_",
        mb.EngineType.Activation: "Activation_",
        mb.EngineType.Pool: "Pool_",
        mb.EngineType.SP: "SP_",
    }

    f = nc.m.functions[0]
    sem_engines = defaultdict(set)
    for blk in f.blocks:
        for ins in blk.instructions:
            si = ins.sync_info
            if si is not None and si.on_update:
                for u in si.on_update:
                    sem_engines[u.ant_name].add(ins.engine)

    incs = defaultdict(int)
    for blk in f.blocks:
        prev_by_engine = {}  # do not move waits across block boundaries
        free_slots = defaultdict(list)  # engine -> earlier 0-wait instructions
        for ins in blk.instructions:
            si = ins.sync_info
            op = str(ins.opcode)
            if si is not None and si.on_wait and len(si.on_wait) > 1 and op != "Drain":
                # 1. strip provably-redundant self-waits
                kept = [
                    w
                    for w in si.on_wait
                    if not (
                        w.ant_name.startswith(eng_pfx.get(ins.engine, "\x00"))
                        and sem_engines.get(w.ant_name) == {ins.engine}
                        and w.wait_value <= incs[w.ant_name]
                    )
                ]
                # 2. spill DMA-queue waits onto earlier same-engine free
                #    slots.  Safe at any distance: queue sems are advanced by
                #    DMA hardware fed by SP-issued copies, and SP waits on no
                #    compute-engine semaphore, so waiting earlier on this
                #    engine cannot form a cycle.
                if len(kept) > 1:
                    dma_w = [w for w in kept if w.ant_name.startswith("DMAHW")]
                    other = [w for w in kept if not w.ant_name.startswith("DMAHW")]
                    while len(other) + len(dma_w) > 1 and dma_w and free_slots[ins.engine]:
                        w = dma_w.pop()
                        carrier = free_slots[ins.engine].pop()
                        csi = carrier.sync_info
                        carrier.sync_info = mb.SyncInfo(
                            on_wait=[w],
                            on_update=list(csi.on_update)
                            if csi is not None and csi.on_update
                            else [],
                        )
                    kept = other + dma_w
                # 3. merge remaining excess with the immediate predecessor
                if len(kept) > 1:
                    prev = prev_by_engine.get(ins.engine)
                    assert prev is not None, f"{ins.name} {op}: no carrier for {kept}"
                    psi = prev.sync_info
                    pw = list(psi.on_wait) if psi is not None and psi.on_wait else []
                    # merge waits per semaphore (max value wins); the carrier
                    # must keep covering its own original waits (they may only
                    # be raised, never dropped or moved later)
                    merged = {}
                    for w in pw + kept:
                        if (
                            w.ant_name not in merged
                            or w.wait_value > merged[w.ant_name].wait_value
                        ):
                            merged[w.ant_name] = w
                    assert len(merged) <= 2, (
                        f"{ins.name} {op}: cannot legalize waits {pw} + {kept}"
                    )
                    carrier_sems = {w.ant_name for w in pw}
                    cw = [w for k, w in merged.items() if k in carrier_sems]
                    tw = [w for k, w in merged.items() if k not in carrier_sems]
                    if not cw:
                        cw, tw = tw[:-1], tw[-1:]
                    elif len(tw) > 1:
                        cw, tw = cw + tw[:-1], tw[-1:]
                    assert len(cw) <= 1 and len(tw) <= 1
                    prev.sync_info = mb.SyncInfo(
                        on_wait=cw,
                        on_update=list(psi.on_update) if psi and psi.on_update else [],
                    )
                    kept = tw
                if len(kept) < len(si.on_wait):
                    ins.sync_info = mb.SyncInfo(on_wait=kept, on_update=si.on_update)
            if si is not None and si.on_update:
                for u in si.on_update:
                    incs[u.ant_name] += u.update_value
            prev_by_engine[ins.engine] = ins
            si = ins.sync_info
            if (si is None or not si.on_wait) and str(ins.opcode) in (
                "Activation",
                "TensorCopy",
                "Ldweights",
                "TensorScalarPtr",
                "TensorScalar",
                "Memset",
            ):
                free_slots[ins.engine].append(ins)

    _legalize_tail_drain(nc, eng_pfx)
    return nc


def _legalize_tail_drain(nc, eng_pfx):
    """The kernel-tail leader Drain collects one wait per engine plus one per
    DMA queue (11 here) -- far over the single wait slot.  Two rewrites:

    - Engine-sem waits are redundant: the all-engine barrier butterfly that
      immediately follows has every engine drain *itself* before gathering,
      which subsumes a foreign wait on that engine's completion count.
    - The DMA-queue waits are load-bearing (queues complete asynchronously;
      nothing else guarantees the final output DMAs have landed).  Keep one
      on the drain and spread the rest over the butterfly instructions that
      have a free wait slot or a vacuous `barrier_* >= 0` wait.  Every such
      slot executes before the final barrier release, so the kernel still
      cannot finish before all queues are flushed; and DMA progress depends
      on no engine, so no added wait can deadlock.
    """
    from concourse import mybir as mb

    f = nc.m.functions[0]
    allins = []
    for blk in f.blocks:
        for ins in blk.instructions:
            allins.append(ins)
    eng_sems = set(eng_pfx.values())

    for idx, ins in enumerate(allins):
        si = ins.sync_info
        if (
            str(ins.opcode) != "Drain"
            or si is None
            or not si.on_wait
            or len(si.on_wait) <= 1
        ):
            continue
        is_eng = lambda w: any(w.ant_name.startswith(p) for p in eng_sems)  # noqa: E731
        dma_waits = [w for w in si.on_wait if not is_eng(w)]
        spill, kept = dma_waits[:-1], dma_waits[-1:]
        for w in spill:
            placed = False
            for nxt in allins[idx + 1 :]:
                nsi = nxt.sync_info
                nw = list(nsi.on_wait) if nsi is not None and nsi.on_wait else []
                vacuous = len(nw) == 1 and nw[0].wait_value == 0
                if len(nw) == 0 or vacuous:
                    nxt.sync_info = mb.SyncInfo(
                        on_wait=[w],
                        on_update=list(nsi.on_update)
                        if nsi is not None and nsi.on_update
                        else [],
                    )
                    placed = True
                    break
            assert placed, f"no tail slot for {w}"
        ins.sync_info = mb.SyncInfo(on_wait=kept, on_update=si.on_update)


def make_weights():
    """Block-diagonal reduction stationaries: wg[:, 64g:64(g+1)] is W_g with
    W_g[p, q] = 1 iff q == 8g + p//16 (sums the 16 C-lanes of each B row)."""
    wg = np.zeros((128, NBLK * B), dtype=np.float32)
    for g in range(NBLK):
        for p in range(128):
            wg[p, B * g + 8 * g + p // 16] = 1.0
    return wg.astype(ml_dtypes.bfloat16)


_CACHE = {}


def _ensure_ntff_hook():
    """Register the axon NTFF profile hook if the image's antenv lacks it."""
    try:
        from antenv import axon_hooks  # noqa: F401

        return
    except ImportError:
        pass
    try:
        import types

        import antenv
        from trn_agent_boot.trn_boot import _ntff_profile_via_ctypes

        hook = _ntff_profile_via_ctypes("/opt/axon/libaxon_pjrt.so")
        mod = types.ModuleType("antenv.axon_hooks")
        state = {"hook": hook}
        mod.set_axon_ntff_profile_hook = lambda h: state.__setitem__("hook", h)
        mod.get_axon_ntff_profile_hook = lambda: state["hook"]
        sys.modules["antenv.axon_hooks"] = mod
        antenv.axon_hooks = mod
    except Exception as e:  # degrade to no tracing
        print(f"ntff hook registration failed: {e}")


def kernel(x: np.ndarray, T: np.ndarray) -> np.ndarray:
    x = np.ascontiguousarray(np.asarray(x, dtype=np.float32))
    t2 = np.ascontiguousarray(np.asarray(T, dtype=np.float32).reshape(A, BC))
    xt = np.ascontiguousarray(x.T)

    if "nc" not in _CACHE:
        _CACHE["nc"] = build_nc()
        _CACHE["wg"] = make_weights()
        _CACHE["id64"] = np.eye(B, dtype=np.float32)
    nc = _CACHE["nc"]

    in_maps = []
    for c in range(NCORES):
        sl = slice(ROWS * c, ROWS * (c + 1))
        in_maps.append(
            {
                "xr": np.ascontiguousarray(x[sl]),
                "xrt": np.ascontiguousarray(xt[:, sl]),
                "xt": xt,
                "t2": t2,
                "wg": _CACHE["wg"],
                "id64": _CACHE["id64"],
            }
        )

    trace = bool(int(os.environ.get("KERNEL_TRACE", "0")))
    if trace:
        _ensure_ntff_hook()
    res = run_bass_kernel_spmd(nc, in_maps, core_ids=list(range(NCORES)), trace=trace)
    if trace:
        _CACHE["last_results"] = res
    out = np.concatenate([r["out"] for r in res.results], axis=0)
    return out.astype(np.float32)


# revision 36
# speedup vs baseline: 1.0071x; 1.0071x over previous
"""Minibatch discrimination 1d kernel for TRN2, 8 NeuronCores.

Reference computation:
    M = (x @ T.reshape(A, B*C)).reshape(N, B, C)        # N=512, A=1024, B=64, C=16
    l1[i,j,b] = sum_c |M[i,b,c] - M[j,b,c]|
    out[i,b]  = sum_j exp(-l1[i,j,b]) - 1
    return concat([x, out], axis=1)                     # (512, 1088)

Distribution: shard the i (row) axis across 8 cores (64 rows each); every core
computes the full M^T = (B*C, N) on-chip (replicated preamble matmul) and the
pairwise reduction for its own rows against all N columns.

Per-core on-chip dataflow, for each of the 64 rows i:
  - DVE:  8x tensor_scalar(|M^T_blk - m_i|) in bf16 (4x perf mode)
  - PE:   8x block-diagonal ones-matmuls reduce the 16 C-lanes per B into
          PSUM l1[64b, 512j] (fp32 accumulate)
  - ACT:  exp(-l1) with free-axis accum_out -> rowsum[64b, 1]
bf16 is numerically safe here: l1 >= ~135 off-diagonal so exp(-l1) underflows
against the diagonal term 1.0 in fp32 regardless of small l1 perturbations.
"""

import os
import sys

import numpy as np

for _p in ("/opt/trn_rl_repo",):
    if _p not in sys.path:
        sys.path.insert(0, _p)

import ml_dtypes  # noqa: E402
from concourse import bass, tile  # noqa: E402
from concourse import mybir  # noqa: E402
from concourse.bass_utils import run_bass_kernel_spmd  # noqa: E402

N, A, B, C = 512, 1024, 64, 16
NCORES = 8
ROWS = N // NCORES  # 64 rows per core
BC = B * C  # 1024
NBLK = BC // 128  # 8 partition blocks of M^T
ABLK = A // 128  # 8 contraction blocks
OUTW = A + B  # 1088
CHUNK = 3  # rows per stationary-reuse chunk (2*CHUNK PSUM banks in flight)

F32 = mybir.dt.float32
BF16 = mybir.dt.bfloat16


def build_nc():
    nc = bass.Bass()
    xr_d = nc.declare_dram_parameter("xr", [ROWS, A], F32, isOutput=False)
    xrt_d = nc.declare_dram_parameter("xrt", [A, ROWS], F32, isOutput=False)
    xt_d = nc.declare_dram_parameter("xt", [A, N], F32, isOutput=False)
    t2_d = nc.declare_dram_parameter("t2", [A, BC], F32, isOutput=False)
    wg_d = nc.declare_dram_parameter("wg", [128, NBLK * B], BF16, isOutput=False)
    id_d = nc.declare_dram_parameter("id64", [B, B], F32, isOutput=False)
    out_d = nc.declare_dram_parameter("out", [ROWS, OUTW], F32, isOutput=True)

    with tile.TileContext(nc) as tc:
        with (
            tc.tile_pool(name="const", bufs=1) as cpool,
            tc.tile_pool(name="work", bufs=2 * CHUNK) as wpool,
            tc.tile_pool(name="pl1", bufs=2 * CHUNK, space="PSUM") as pl1,
        ):
            # ---- output staging: x rows land in SBUF, cols appended later,
            # one single DMA writes the full [64, 1088] block ----
            out_sb = cpool.tile([ROWS, OUTW], F32)
            nc.sync.dma_start(out=out_sb[:, 0:A], in_=xr_d[:])
            # in-place touches: absorb the xr-load DMA wait into ACT's clock
            # and order the final output DMA after it (single wait slot).
            # The second touch carries no wait of its own -- it is a spare
            # wait slot for _legalize_waits to park the output DMA's
            # queue-ring wait on.
            nc.scalar.copy(out_sb[:, 0:1], out_sb[:, 0:1])
            nc.scalar.copy(out_sb[:, 1:2], out_sb[:, 1:2])

            # ---- load + cast inputs ----
            t2_f = cpool.tile([128, ABLK, BC], F32)
            t2_b = cpool.tile([128, ABLK, BC], BF16)
            xt_f = cpool.tile([128, ABLK, N], F32)
            xt_b = cpool.tile([128, ABLK, N], BF16)
            xrt_f = cpool.tile([128, ABLK, ROWS], F32)
            xrt_b = cpool.tile([128, ABLK, ROWS], BF16)
            wg_s = cpool.tile([128, NBLK * B], BF16)
            id_s = cpool.tile([B, B], F32)

            # ---- PE HAM warm-up: the tensor engine is idle during the
            # input DMA phase and would start the preamble matmuls at the
            # cold 1.2 GHz clock gate (K=4/8).  ~6.5us of dummy matmuls on a
            # memset tile keeps the activity monitor busy so everything that
            # follows runs at 2.4 GHz.  No added critical path: the real
            # matmuls wait on DMA + casts (~15us) anyway.
            warm = cpool.tile([128, 128], BF16)
            nc.vector.memset(warm[:], 1.0)
            pwarm = pl1.tile([128, 128], F32, tag="l1")
            for _ in range(60):
                nc.tensor.matmul(pwarm[:], warm[:], warm[:], start=True, stop=True)

            nc.sync.dma_start(out=wg_s[:], in_=wg_d[:])
            nc.sync.dma_start(out=id_s[:], in_=id_d[:])
            for k in range(ABLK):
                nc.sync.dma_start(out=t2_f[:, k, :], in_=t2_d[128 * k : 128 * (k + 1), :])
                nc.sync.dma_start(out=xt_f[:, k, :], in_=xt_d[128 * k : 128 * (k + 1), :])
                nc.sync.dma_start(out=xrt_f[:, k, :], in_=xrt_d[128 * k : 128 * (k + 1), :])
            for k in range(ABLK):
                nc.vector.tensor_copy(t2_b[:, k, :], t2_f[:, k, :])
                nc.vector.tensor_copy(xt_b[:, k, :], xt_f[:, k, :])
                nc.vector.tensor_copy(xrt_b[:, k, :], xrt_f[:, k, :])

            # ---- M^T = t2^T @ x^T : [BC, N] in 8 partition blocks ----
            mt_b = cpool.tile([128, NBLK, N], BF16)
            mi_h = cpool.tile([128, NBLK, ROWS], BF16)
            mi_b = cpool.tile([128, NBLK, ROWS], F32)
            mi_n = cpool.tile([128, NBLK, ROWS], F32)  # negated, ACT abs bias
            with tc.tile_pool(name="ppre", bufs=1, space="PSUM") as ppre:
              for g in range(NBLK):
                  pm = ppre.tile([128, N], F32, tag="pmt")
                  for k in range(ABLK):
                      nc.tensor.matmul(
                          pm[:],
                          t2_b[:, k, 128 * g : 128 * (g + 1)],
                          xt_b[:, k, :],
                          start=(k == 0),
                          stop=(k == ABLK - 1),
                      )
                  # split psum->sbuf cast copies between ACT and DVE
                  if g % 2 == 0:
                      nc.scalar.copy(mt_b[:, g, :], pm[:])
                  else:
                      nc.vector.tensor_copy(mt_b[:, g, :], pm[:])
                  pmi = ppre.tile([128, ROWS], F32, tag="pmi")
                  for k in range(ABLK):
                      nc.tensor.matmul(
                          pmi[:],
                          t2_b[:, k, 128 * g : 128 * (g + 1)],
                          xrt_b[:, k, :],
                          start=(k == 0),
                          stop=(k == ABLK - 1),
                      )
                  # round mi through bf16 so the diagonal |mt_b[:, i] - mi| is
                  # exactly zero (mt_b is bf16-rounded)
                  nc.vector.tensor_copy(mi_h[:, g, :], pmi[:])
                  nc.vector.tensor_copy(mi_b[:, g, :], mi_h[:, g, :])
                  nc.vector.tensor_scalar(
                      mi_n[:, g, :], mi_b[:, g, :], -1.0, None, op0=mybir.AluOpType.mult
                  )

            # ---- main pairwise loop over this core's rows ----
            # |mt - mi| per block: DVE handles ND blocks as subtract (4x perf
            # mode) + one batched uint16 sign-clear AND (4x); ACT handles the
            # rest as a single Abs(x + (-mi)) activation pass per block.
            # Rows are processed in chunks of CHUNK; within a chunk the
            # reduction matmuls run g-outer so each stationary W_g is loaded
            # once per chunk instead of once per matmul (LDWEIGHTS amortize).
            ND = int(os.environ.get("KERNEL_ND", "6"))
            rowacc = cpool.tile([B, ROWS], F32)
            for i0 in range(0, ROWS, CHUNK):
                csz = min(CHUNK, ROWS - i0)
                dvs, das, l1s = [], [], []
                for i in range(i0, i0 + csz):
                    dv = wpool.tile([128, ND, N], BF16, tag="dv")
                    da = None
                    if ND < NBLK:
                        da = wpool.tile([128, NBLK - ND, N], BF16, tag="da")
                    for g in range(ND):
                        nc.vector.tensor_scalar(
                            dv[:, g, :],
                            mt_b[:, g, :],
                            mi_b[:, g, i : i + 1],
                            None,
                            op0=mybir.AluOpType.subtract,
                        )
                    nc.vector.tensor_scalar(
                        dv[:].bitcast(mybir.dt.uint16),
                        dv[:].bitcast(mybir.dt.uint16),
                        0x7FFF,
                        None,
                        op0=mybir.AluOpType.bitwise_and,
                    )
                    for g in range(ND, NBLK):
                        nc.scalar.activation(
                            da[:, g - ND, :],
                            mt_b[:, g, :],
                            mybir.ActivationFunctionType.Abs,
                            bias=mi_n[:, g, i : i + 1],
                            scale=1.0,
                        )
                    dvs.append(dv)
                    das.append(da)
                    l1 = pl1.tile([B, N], F32, tag="l1")
                    l1s.append(l1)
                for g in range(NBLK):
                    for ic in range(csz):
                        src = dvs[ic][:, g, :] if g < ND else das[ic][:, g - ND, :]
                        nc.tensor.matmul(
                            l1s[ic][:],
                            wg_s[:, B * g : B * (g + 1)],
                            src,
                            start=(g == 0),
                            stop=(g == NBLK - 1),
                        )
                for ic in range(csz):
                    i = i0 + ic
                    et = wpool.tile([B, N], BF16, tag="e")
                    nc.scalar.activation(
                        et[:],
                        l1s[ic][:],
                        mybir.ActivationFunctionType.Exp,
                        scale=-1.0,
                        accum_out=rowacc[:, i : i + 1],
                    )

            # ---- finalize: subtract diagonal term, transpose, store ----
            occ = cpool.tile([B, ROWS], F32)
            nc.vector.tensor_scalar(
                occ[:], rowacc[:], 1.0, None, op0=mybir.AluOpType.subtract
            )
            pt = pl1.tile([ROWS, B], F32, tag="l1")
            nc.tensor.transpose(pt[:], occ[:], id_s[:])
            nc.scalar.copy(out_sb[:, A:OUTW], pt[:])
            nc.scalar.dma_start(out=out_d[:], in_=out_sb[:])

    if not os.environ.get("KERNEL_NO_STRIP"):
        _legalize_waits(nc)
    return nc


def _legalize_waits(nc):
    """walrus rejects compute instructions (TensorScalarPtr / Activation /
    Matmult / ...) with more than one sync-wait: their trn2 ISA structs have
    a single wait slot.  Tile sometimes emits 2.  Two safe rewrites:

    1. Strip self-waits that are provably redundant: the waited semaphore is
       incremented exclusively by instructions of the waiter's own engine
       (verified globally) and the wait value is <= the number of increments
       issued earlier in this engine's strictly in-order stream, so the wait
       is always satisfied by engine ordering alone.

    2. Move an excess wait onto the *immediately preceding* instruction of
       the same engine (typically the Ldweights in front of a Matmult) when
       that instruction has a free wait slot.  Waiting earlier in the same
       in-order stream is strictly more conservative, and since there is no
       same-engine instruction between carrier and target, no foreign
       producer chain can depend on an instruction in that window (deadlock
       impossible).
    """
    from collections import defaultdict

    from concourse import mybir as mb

    # Engine-owned semaphore name prefixes.  DMAHW*/DMASW* sems increment
    # asynchronously at DMA *completion*, not at instruction retirement on
    # the issuing engine, so they are never "self" sems.
    eng_pfx = {
        mb.EngineType.PE: "PE_",
        mb.EngineType.DVE: "DVE_",
        mb.EngineType.Activation: "Activation_",
        mb.EngineType.Pool: "Pool_",
        mb.EngineType.SP: "SP_",
    }

    f = nc.m.functions[0]
    sem_engines = defaultdict(set)
    for blk in f.blocks:
        for ins in blk.instructions:
            si = ins.sync_info
            if si is not None and si.on_update:
                for u in si.on_update:
                    sem_engines[u.ant_name].add(ins.engine)

    incs = defaultdict(int)
    for blk in f.blocks:
        prev_by_engine = {}  # do not move waits across block boundaries
        free_slots = defaultdict(list)  # engine -> earlier 0-wait instructions
        for ins in blk.instructions:
            si = ins.sync_info
            op = str(ins.opcode)
            if si is not None and si.on_wait and len(si.on_wait) > 1 and op != "Drain":
                # 1. strip provably-redundant self-waits
                kept = [
                    w
                    for w in si.on_wait
                    if not (
                        w.ant_name.startswith(eng_pfx.get(ins.engine, "\x00"))
                        and sem_engines.get(w.ant_name) == {ins.engine}
                        and w.wait_value <= incs[w.ant_name]
                    )
                ]
                # 2. spill DMA-queue waits onto earlier same-engine free
                #    slots.  Safe at any distance: queue sems are advanced by
                #    DMA hardware fed by SP-issued copies, and SP waits on no
                #    compute-engine semaphore, so waiting earlier on this
                #    engine cannot form a cycle.
                if len(kept) > 1:
                    dma_w = [w for w in kept if w.ant_name.startswith("DMAHW")]
                    other = [w for w in kept if not w.ant_name.startswith("DMAHW")]
                    while len(other) + len(dma_w) > 1 and dma_w and free_slots[ins.engine]:
                        w = dma_w.pop()
                        carrier = free_slots[ins.engine].pop()
                        csi = carrier.sync_info
                        carrier.sync_info = mb.SyncInfo(
                            on_wait=[w],
                            on_update=list(csi.on_update)
                            if csi is not None and csi.on_update
                            else [],
                        )
                    kept = other + dma_w
                # 3. merge remaining excess with the immediate predecessor
                if len(kept) > 1:
                    prev = prev_by_engine.get(ins.engine)
                    assert prev is not None, f"{ins.name} {op}: no carrier for {kept}"
                    psi = prev.sync_info
                    pw = list(psi.on_wait) if psi is not None and psi.on_wait else []
                    # merge waits per semaphore (max value wins); the carrier
                    # must keep covering its own original waits (they may only
                    # be raised, never dropped or moved later)
                    merged = {}
                    for w in pw + kept:
                        if (
                            w.ant_name not in merged
                            or w.wait_value > merged[w.ant_name].wait_value
                        ):
                            merged[w.ant_name] = w
                    assert len(merged) <= 2, (
                        f"{ins.name} {op}: cannot legalize waits {pw} + {kept}"
                    )
                    carrier_sems = {w.ant_name for w in pw}
                    cw = [w for k, w in merged.items() if k in carrier_sems]
                    tw = [w for k, w in merged.items() if k not in carrier_sems]
                    if not cw:
                        cw, tw = tw[:-1], tw[-1:]
                    elif len(tw) > 1:
                        cw, tw = cw + tw[:-1], tw[-1:]
                    assert len(cw) <= 1 and len(tw) <= 1
                    prev.sync_info = mb.SyncInfo(
                        on_wait=cw,
                        on_update=list(psi.on_update) if psi and psi.on_update else [],
                    )
                    kept = tw
                if len(kept) < len(si.on_wait):
                    ins.sync_info = mb.SyncInfo(on_wait=kept, on_update=si.on_update)
            if si is not None and si.on_update:
                for u in si.on_update:
                    incs[u.ant_name] += u.update_value
            prev_by_engine[ins.engine] = ins
            si = ins.sync_info
            if (si is None or not si.on_wait) and str(ins.opcode) in (
                "Activation",
                "TensorCopy",
                "Ldweights",
                "TensorScalarPtr",
                "TensorScalar",
                "Memset",
            ):
                free_slots[ins.engine].append(ins)

    _legalize_tail_drain(nc, eng_pfx)
    return nc


def _legalize_tail_drain(nc, eng_pfx):
    """The kernel-tail leader Drain collects one wait per engine plus one per
    DMA queue (11 here) -- far over the single wait slot.  Two rewrites:

    - Engine-sem waits are redundant: the all-engine barrier butterfly that
      immediately follows has every engine drain *itself* before gathering,
      which subsumes a foreign wait on that engine's completion count.
    - The DMA-queue waits are load-bearing (queues complete asynchronously;
      nothing else guarantees the final output DMAs have landed).  Keep one
      on the drain and spread the rest over the butterfly instructions that
      have a free wait slot or a vacuous `barrier_* >= 0` wait.  Every such
      slot executes before the final barrier release, so the kernel still
      cannot finish before all queues are flushed; and DMA progress depends
      on no engine, so no added wait can deadlock.
    """
    from concourse import mybir as mb

    f = nc.m.functions[0]
    allins = []
    for blk in f.blocks:
        for ins in blk.instructions:
            allins.append(ins)
    eng_sems = set(eng_pfx.values())

    for idx, ins in enumerate(allins):
        si = ins.sync_info
        if (
            str(ins.opcode) != "Drain"
            or si is None
            or not si.on_wait
            or len(si.on_wait) <= 1
        ):
            continue
        is_eng = lambda w: any(w.ant_name.startswith(p) for p in eng_sems)  # noqa: E731
        dma_waits = [w for w in si.on_wait if not is_eng(w)]
        spill, kept = dma_waits[:-1], dma_waits[-1:]
        for w in spill:
            placed = False
            for nxt in allins[idx + 1 :]:
                nsi = nxt.sync_info
                nw = list(nsi.on_wait) if nsi is not None and nsi.on_wait else []
                vacuous = len(nw) == 1 and nw[0].wait_value == 0
                if len(nw) == 0 or vacuous:
                    nxt.sync_info = mb.SyncInfo(
                        on_wait=[w],
                        on_update=list(nsi.on_update)
                        if nsi is not None and nsi.on_update
                        else [],
                    )
                    placed = True
                    break
            assert placed, f"no tail slot for {w}"
        ins.sync_info = mb.SyncInfo(on_wait=kept, on_update=si.on_update)


def make_weights():
    """Block-diagonal reduction stationaries: wg[:, 64g:64(g+1)] is W_g with
    W_g[p, q] = 1 iff q == 8g + p//16 (sums the 16 C-lanes of each B row)."""
    wg = np.zeros((128, NBLK * B), dtype=np.float32)
    for g in range(NBLK):
        for p in range(128):
            wg[p, B * g + 8 * g + p // 16] = 1.0
    return wg.astype(ml_dtypes.bfloat16)


_CACHE = {}


def _ensure_ntff_hook():
    """Register the axon NTFF profile hook if the image's antenv lacks it."""
    try:
        from antenv import axon_hooks  # noqa: F401

        return
    except ImportError:
        pass
    try:
        import types

        import antenv
        from trn_agent_boot.trn_boot import _ntff_profile_via_ctypes

        hook = _ntff_profile_via_ctypes("/opt/axon/libaxon_pjrt.so")
        mod = types.ModuleType("antenv.axon_hooks")
        state = {"hook": hook}
        mod.set_axon_ntff_profile_hook = lambda h: state.__setitem__("hook", h)
        mod.get_axon_ntff_profile_hook = lambda: state["hook"]
        sys.modules["antenv.axon_hooks"] = mod
        antenv.axon_hooks = mod
    except Exception as e:  # degrade to no tracing
        print(f"ntff hook registration failed: {e}")


def kernel(x: np.ndarray, T: np.ndarray) -> np.ndarray:
    x = np.ascontiguousarray(np.asarray(x, dtype=np.float32))
    t2 = np.ascontiguousarray(np.asarray(T, dtype=np.float32).reshape(A, BC))
    xt = np.ascontiguousarray(x.T)

    if "nc" not in _CACHE:
        _CACHE["nc"] = build_nc()
        _CACHE["wg"] = make_weights()
        _CACHE["id64"] = np.eye(B, dtype=np.float32)
    nc = _CACHE["nc"]

    in_maps = []
    for c in range(NCORES):
        sl = slice(ROWS * c, ROWS * (c + 1))
        in_maps.append(
            {
                "xr": np.ascontiguousarray(x[sl]),
                "xrt": np.ascontiguousarray(xt[:, sl]),
                "xt": xt,
                "t2": t2,
                "wg": _CACHE["wg"],
                "id64": _CACHE["id64"],
            }
        )

    trace = bool(int(os.environ.get("KERNEL_TRACE", "0")))
    if trace:
        _ensure_ntff_hook()
    res = run_bass_kernel_spmd(nc, in_maps, core_ids=list(range(NCORES)), trace=trace)
    if trace:
        _CACHE["last_results"] = res
    out = np.concatenate([r["out"] for r in res.results], axis=0)
    return out.astype(np.float32)
